# revision 43
# baseline (speedup 1.0000x reference)
"""Self-contained distributed AFGCN kernel for 8 TRN2 NeuronCores.

kernel(**inputs) takes the FULL unsharded inputs (as produced by the
problem's setup_inputs) and returns the FULL [100000] float32 output.

Pipeline per core (SPMD, one compiled graph):
  - full padded node-feature replica in DRAM (4 int16-indexed banks),
    per-edge dma_gather of source rows;
  - scatter-add via one-hot (DVE is_equal vs plane-offset iota) matmuls
    accumulated in PSUM, bf16 operands / fp32 accumulation;
  - dense GraphConv update in feature-transposed layout (PE + ACT + DVE);
  - AllGather rebuilds the replica between layers;
  - fc head -> per-core output shard, host concatenates.
"""
import numpy as np
import ml_dtypes
from contextlib import ExitStack
from dataclasses import dataclass


@dataclass
class Cfg:
    N: int = 100000          # real nodes
    D: int = 64
    L: int = 4
    NC: int = 8
    NSH: int = 12544         # nodes per core (NBLK*128)
    GB: int = 14             # dst blocks per super-group
    NBANK: int = 4
    PADM: int = 8            # group cap quantum (full-col matmuls + plane-masked one-hot)
    MAXI: int = 1024         # idxs per dma_gather call
    DCH: int = 448           # dense node-chunk cols

    @property
    def NP(self):
        return self.NC * self.NSH

    @property
    def NBLK(self):
        return self.NSH // 128

    @property
    def NG(self):
        return self.NBLK // self.GB

    @property
    def BANK(self):
        return self.NP // self.NBANK


FULL = Cfg()
SMALL = Cfg(N=2000, NSH=256, GB=2, DCH=128)  # NP=2048, NBLK=2, NG=1


def roundup(x, m):
    return -(-x // m) * m


def ag_sets(cfg):
    """Contiguous group sets, one AllGather per set per layer."""
    return [(g,) for g in range(cfg.NG)]


def rowid_map(cfg):
    """node id -> gather-table row ([ag_set][core][group-in-set][slot])."""
    v = np.arange(cfg.NP, dtype=np.int64)
    gcols = cfg.GB * 128
    c_v, s_v = v // cfg.NSH, v % cfg.NSH
    g_v, r_v = s_v // gcols, s_v % gcols
    rid = np.empty(cfg.NP, np.int64)
    base = 0
    for s in ag_sets(cfg):
        size = len(s) * gcols
        for gi, g in enumerate(s):
            m = g_v == g
            rid[m] = base + c_v[m] * size + gi * gcols + r_v[m]
        base += cfg.NC * size
    return rid


def make_plan(src_g, dst_g, cfg):
    """SPMD-uniform skeleton + per-core gather/onehot data."""
    cN = cfg
    rid = rowid_map(cN)
    per = []
    sizes = np.zeros((cN.NC, cN.NBLK, cN.NBANK), np.int64)
    for c in range(cN.NC):
        lo, hi = c * cN.NSH, (c + 1) * cN.NSH
        m = (dst_g >= lo) & (dst_g < hi)
        src, dst = rid[src_g[m]], dst_g[m] - lo
        blk = dst // 128
        bank = src // cN.BANK
        d = {}
        for b in range(cN.NBLK):
            mb = blk == b
            sb, db, kb = src[mb], dst[mb], bank[mb]
            for k in range(cN.NBANK):
                mk = kb == k
                d[(b, k)] = (sb[mk], db[mk])
                sizes[c, b, k] = mk.sum()
        per.append(d)

    caps = np.maximum(roundup(sizes.max(axis=0), cN.PADM), cN.PADM)

    runs, chunks = [], []
    S = 0
    for g in range(cN.NG):
        for k in range(cN.NBANK):
            run_start = S
            for b in range(g * cN.GB, (g + 1) * cN.GB):
                grp_start = S
                S += int(caps[b, k])
                s0 = grp_start
                while s0 < S:
                    s1 = min(S, (s0 // 128 + 1) * 128)
                    chunks.append(dict(col=s0 // 128, p0=s0 % 128,
                                       p1=s1 - (s0 // 128) * 128,
                                       block=b, g=g, bank=k))
                    s0 = s1
            pad = -(S - run_start) % 128
            S += pad
            size = S - run_start
            calls = []
            off = 0
            while off < size:
                ni = min(cN.MAXI, size - off)
                calls.append((run_start + off, ni))
                off += ni
            runs.append(dict(g=g, bank=k, start=run_start, size=size,
                             calls=calls))
    # start/stop flags per (block, bank) group, in col order
    by_bb = {}
    for ch in chunks:
        by_bb.setdefault((ch["block"], ch["bank"]), []).append(ch)
    for _, lst in by_bb.items():
        lst.sort(key=lambda ch: (ch["col"], ch["p0"]))
        for i, ch in enumerate(lst):
            ch["start"] = i == 0
            ch["stop"] = i == len(lst) - 1

    # merged one-hot windows: per col, consecutive blocks within one PSUM
    # bank (4 blocks = 512 f32 cols); all 4 bank-k runs accumulate into one
    # PSUM group per (g, psum_bank) -> start on first toucher, stop on last
    by_gk_col = {}
    for ch in chunks:
        by_gk_col.setdefault((ch["g"], ch["bank"], ch["col"]), []).append(
            ch["block"])
    windows = []
    for run in runs:
        g, k = run["g"], run["bank"]
        c0 = run["start"] // 128
        for col in range(c0, (run["start"] + run["size"]) // 128):
            blocks = sorted(by_gk_col.get((g, k, col), []))
            if not blocks:
                continue
            b = blocks[0]
            while b <= blocks[-1]:
                rel = b - g * cN.GB
                bank_end = g * cN.GB + (rel // 4 + 1) * 4
                be = min(blocks[-1], bank_end - 1)
                windows.append(dict(g=g, k=k, col=col, b0=b, nb=be - b + 1))
                b = be + 1
    # stop flags for both within-group k orders (k asc on even layers,
    # k desc on odd layers -- chosen so gather banks are ready in
    # consumption order at layer boundaries)
    for rev, stop_key in ((False, "stop"), (True, "stop_r")):
        order = {}
        seq = sorted(
            range(len(windows)),
            key=lambda i: (windows[i]["g"],
                           -windows[i]["k"] if rev else windows[i]["k"],
                           windows[i]["col"]))
        for pos, i in enumerate(seq):
            w = windows[i]
            key = (w["g"], (w["b0"] - w["g"] * cN.GB) // 4)
            order.setdefault(key, [pos, pos])[1] = pos
        for pos, i in enumerate(seq):
            w = windows[i]
            first, last = order[(w["g"], (w["b0"] - w["g"] * cN.GB) // 4)]
            w[stop_key] = pos == last
    skeleton = dict(caps=caps, runs=runs, chunks=chunks, windows=windows, S=S)

    per_core = []
    run_of = {(r["g"], r["bank"]): r for r in runs}
    for c in range(cN.NC):
        slots_src = np.zeros(S, np.int64)
        dstloc = np.full(S, -1.0, np.float32)
        for g in range(cN.NG):
            for k in range(cN.NBANK):
                run = run_of[(g, k)]
                pos = run["start"]
                for b in range(g * cN.GB, (g + 1) * cN.GB):
                    sb, db = per[c][(b, k)]
                    n = len(sb)
                    slots_src[pos : pos + n] = sb
                    slots_src[pos + n : pos + int(caps[b, k])] = k * cN.BANK
                    dstloc[pos : pos + n] = (db - b * 128).astype(np.float32) \
                        + 128.0 * (b % 16)
                    pos += int(caps[b, k])
                slots_src[pos : run["start"] + run["size"]] = k * cN.BANK
        idxflat = (slots_src % cN.BANK).astype(np.int16)
        idx16 = np.zeros((128, S // 16), np.int16)
        for run in runs:
            for (off, ni) in run["calls"]:
                seg = idxflat[off : off + ni]
                w = seg.reshape(ni // 16, 16).T
                idx16[:, off // 16 : (off + ni) // 16] = np.tile(w, (8, 1))
        dstloc_t = dstloc.reshape(S // 128, 128).T.copy()
        per_core.append(dict(idx16=idx16, dstloc=dstloc_t,
                             slots_src=slots_src, dstloc_flat=dstloc))
    skeleton["rowid"] = rid
    return skeleton, per_core


def plan_forward_numpy(inputs, cfg, skeleton, per_core):
    x = np.asarray(inputs["x"], np.float32)
    W_rel = np.asarray(inputs["W_rel"], np.float32)
    b_rel = np.asarray(inputs["b_rel"], np.float32)
    W_root = np.asarray(inputs["W_root"], np.float32)
    fc_w = np.asarray(inputs["fc_w"], np.float32)
    fc_b = np.asarray(inputs["fc_b"], np.float32)
    hp = np.zeros((cfg.NP, cfg.D), np.float32)
    hp[: cfg.N] = x
    h0 = hp.copy()
    rid = skeleton["rowid"]
    for l in range(cfg.L):
        tab = np.empty_like(hp)
        tab[rid] = hp
        agg = np.zeros((cfg.NP, cfg.D), np.float32)
        for c in range(cfg.NC):
            pc = per_core[c]
            msgs = tab[pc["slots_src"]]
            dl = pc["dstloc_flat"]
            for ch in skeleton["chunks"]:
                s0 = ch["col"] * 128
                s1 = s0 + 128
                pl = ch["block"] % 16
                onehot = dl[s0:s1, None] == (np.arange(128)[None, :] + 128.0 * pl)
                base = c * cfg.NSH + ch["block"] * 128
                agg[base : base + 128] += onehot.astype(np.float32).T @ msgs[s0:s1]
        z = agg @ W_rel[l] + b_rel[l] + hp @ W_root[l]
        hp = np.maximum(z, 0.0) + h0
    out = hp @ fc_w + fc_b
    return out[: cfg.N, 0]


def build_nc(cfg, skeleton, fc_b_val):
    import concourse.bass as bass
    import concourse.tile as tile
    from concourse import bacc, mybir

    STAGE = 5

    f32 = mybir.dt.float32
    f16 = mybir.dt.float16
    i16 = mybir.dt.int16
    cN = cfg
    S = skeleton["S"]
    runs, chunks = skeleton["runs"], skeleton["chunks"]
    windows = skeleton["windows"]
    run_of = {(r["g"], r["bank"]): r for r in runs}
    sets = ag_sets(cfg)

    nc = bacc.Bacc("TRN2", target_bir_lowering=False, debug=False,
                   num_devices=cN.NC, num_swdge_queues=4,
                   dynamic_dma_scratch_size=32768)

    t_pre = nc.dram_tensor("pre", [128, (S // 128) * cN.D], f16,
                           kind="ExternalInput")
    t_xT = nc.dram_tensor("xT", [cN.D, cN.NSH], f32, kind="ExternalInput")
    t_idx = nc.dram_tensor("idx", [128, S // 16], i16, kind="ExternalInput")
    t_dstloc = nc.dram_tensor("dstloc", [128, S // 128], f16, kind="ExternalInput")
    t_iota = nc.dram_tensor("iota", [128, 31 * 128], f16, kind="ExternalInput")
    t_ident = nc.dram_tensor("ident", [128, 128], f32, kind="ExternalInput")
    t_Wr = nc.dram_tensor("Wr", [cN.L, cN.D, cN.D], f16, kind="ExternalInput")
    t_Wo = nc.dram_tensor("Wo", [cN.L, cN.D, cN.D], f16, kind="ExternalInput")
    t_br = nc.dram_tensor("br", [cN.L, cN.D], f32, kind="ExternalInput")
    t_fcw = nc.dram_tensor("fcw", [cN.D, 1], f32, kind="ExternalInput")
    t_fcb = nc.dram_tensor("fcb", [1, 1], f32, kind="ExternalInput")
    t_out = nc.dram_tensor("out", [1, cN.NSH], f32, kind="ExternalOutput")

    # fp16 gather tables with 256B-padded rows (gather elem = 128 fp16)
    t_inb, t_outb = [], []
    for l in range(cN.L - 1):
        t_inb.append(nc.dram_tensor(f"inb{l}", [cN.NSH, 2 * cN.D], f16))
        t_outb.append(nc.dram_tensor(f"outb{l}", [cN.NP, 2 * cN.D], f16,
                                     addr_space="Shared"))

    GCOLS = cN.GB * 128

    with tile.TileContext(nc) as tc, ExitStack() as ctx:
        c_pool = ctx.enter_context(tc.tile_pool(name="const", bufs=1))
        h_pool = ctx.enter_context(tc.tile_pool(name="hbuf", bufs=1))
        msg_pool = ctx.enter_context(tc.tile_pool(name="msg", bufs=7))
        idx_pool = ctx.enter_context(tc.tile_pool(name="idxp", bufs=6))
        oh_pool = ctx.enter_context(tc.tile_pool(name="oh", bufs=12))
        st_pool = ctx.enter_context(tc.tile_pool(name="stage", bufs=2))
        ep_pool = ctx.enter_context(tc.tile_pool(name="epi", bufs=2))
        h0_pool = ctx.enter_context(tc.tile_pool(name="h0sl", bufs=2))
        ps_agg = ctx.enter_context(tc.tile_pool(name="psagg", bufs=1, space="PSUM"))
        ps_z = ctx.enter_context(tc.tile_pool(name="psz", bufs=2, space="PSUM"))
        ps_t = ctx.enter_context(tc.tile_pool(name="pst", bufs=1, space="PSUM"))
        ps_fc = ctx.enter_context(tc.tile_pool(name="psfc", bufs=1, space="PSUM"))

        dstloc_t = c_pool.tile([128, S // 128], f16, tag="dstloc")
        nc.sync.dma_start(out=dstloc_t[:], in_=t_dstloc[:])
        iota_t = c_pool.tile([128, 31 * 128], f16, tag="iota")
        nc.sync.dma_start(out=iota_t[:], in_=t_iota[:])
        ident_t = c_pool.tile([128, 128], f32, tag="ident")
        nc.sync.dma_start(out=ident_t[:], in_=t_ident[:])
        wr_t, wo_t, br_t = [], [], []
        for l in range(cN.L):
            w1 = c_pool.tile([cN.D, cN.D], f16, tag=f"wr{l}")
            nc.sync.dma_start(out=w1[:], in_=t_Wr[l, :, :])
            wr_t.append(w1)
            w2 = c_pool.tile([cN.D, cN.D], f16, tag=f"wo{l}")
            nc.sync.dma_start(out=w2[:], in_=t_Wo[l, :, :])
            wo_t.append(w2)
            bb = c_pool.tile([cN.D, 1], f32, tag=f"br{l}")
            nc.sync.dma_start(out=bb[:], in_=t_br[l, :, None])
            br_t.append(bb)
        fcw_t = c_pool.tile([cN.D, 1], f32, tag="fcw")
        nc.sync.dma_start(out=fcw_t[:], in_=t_fcw[:])
        fcb_t = c_pool.tile([1, 1], f32, tag="fcb")
        nc.sync.dma_start(out=fcb_t[:], in_=t_fcb[:])
        zero_t = c_pool.tile([128, 512], f16, tag="zero")
        nc.vector.memset(zero_t[:], 0.0)

        gq = [0]
        hT = h_pool.tile([cN.D, cN.NSH], f32, tag="hT")
        nc.sync.dma_start(out=hT[:], in_=t_xT[:])

        # AllGather emissions are deferred by 2 groups: collectives issue
        # from the GpSimd queue, and an AG whose input isn't ready yet
        # blocks every gather queued behind it (killing gather lookahead)
        agq = []

        def flush_ag(keep):
            while len(agq) > keep:
                nc.gpsimd.collective_compute(**agq.pop(0))

        for l in range(cN.L):
            E = cN.D if l == 0 else 2 * cN.D   # msg slot width (fp16 elems)
            # alternate group order: descending layers start with the groups
            # whose gather banks were AllGathered last in the previous layer
            grange = range(cN.NG) if l % 2 == 0 else range(cN.NG - 1, -1, -1)
            flush_ag(0)
            for g in grange:
                flush_ag(1)
                # 4 full PSUM banks; zeroed via start=True dummy matmuls so
                # the variable-width window matmuls are pure accumulates
                agg_ps = ps_agg.tile([cN.D, 2048], f32, tag="agg")
                for bk in range(4):
                    nc.tensor.matmul(
                        out=agg_ps[:, bk * 512 : (bk + 1) * 512],
                        lhsT=zero_t[:, : cN.D],
                        rhs=zero_t[:],
                        start=True, stop=False)
                krange = (range(cN.NBANK) if l % 2 == 0
                          else range(cN.NBANK - 1, -1, -1))
                for k in krange:
                    run = run_of[(g, k)]
                    ncols = run["size"] // 128
                    rc0 = run["start"] // 128
                    mt = msg_pool.tile([128, ncols * E], f16, tag="msg")
                    if l == 0:
                        nc.sync.dma_start(
                            out=mt[:],
                            in_=t_pre[:, rc0 * cN.D : (rc0 + ncols) * cN.D])
                    else:
                        table = t_outb[l - 1]
                        bank_ap = table[k * cN.BANK : (k + 1) * cN.BANK, :]
                        for (off, ni) in run["calls"]:
                            it = idx_pool.tile([128, ni // 16], i16, tag="idxt")
                            nc.sync.dma_start(
                                out=it[:],
                                in_=t_idx[:, off // 16 : (off + ni) // 16])
                            o0 = (off - run["start"]) // 128
                            nc.gpsimd.dma_gather(
                                out_ap=mt[:, o0 * E : (o0 + ni // 128) * E]
                                    .rearrange("p (c d) -> p c d", d=E),
                                in_ap=bank_ap,
                                idxs_ap=it[:],
                                num_idxs=ni,
                                num_idxs_reg=ni,
                                elem_size=E,
                                queue_num=gq[0] % 4,
                            )
                            gq[0] += 1
                    for w in [x for x in windows
                              if x["g"] == g and x["k"] == k]:
                        rel = w["col"] - rc0
                        W = w["nb"] * 128
                        bcol = (w["b0"] - g * cN.GB) * 128
                        pl = w["b0"] % 16
                        oh = oh_pool.tile([128, W], f16, tag="oh")
                        nc.vector.tensor_tensor(
                            out=oh[:],
                            in0=dstloc_t[:, w["col"] : w["col"] + 1]
                                .to_broadcast([128, W]),
                            in1=iota_t[:, pl * 128 : pl * 128 + W],
                            op=mybir.AluOpType.is_equal,
                        )
                        nc.tensor.matmul(
                            out=agg_ps[:, bcol : bcol + W],
                            lhsT=mt[:, rel * E : rel * E + cN.D],
                            rhs=oh[:],
                            start=False,
                            stop=w["stop" if l % 2 == 0 else "stop_r"],
                        )
                gb0 = g * GCOLS
                if STAGE >= 4:
                    h0sl = h0_pool.tile([cN.D, GCOLS], f32, tag="h0sl")
                    nc.sync.dma_start(out=h0sl[:],
                                      in_=t_xT[:, gb0 : gb0 + GCOLS])
                for j in range(GCOLS // cN.DCH if STAGE >= 4 else 0):
                    cl = gb0 + j * cN.DCH
                    aggb = ep_pool.tile([cN.D, cN.DCH], f16, tag="aggb")
                    nc.scalar.activation(
                        out=aggb[:], in_=agg_ps[:, j * cN.DCH : (j + 1) * cN.DCH],
                        func=mybir.ActivationFunctionType.Identity)
                    hTb = ep_pool.tile([cN.D, cN.DCH], f16, tag="hTb")
                    nc.scalar.activation(
                        out=hTb[:], in_=hT[:, cl : cl + cN.DCH],
                        func=mybir.ActivationFunctionType.Identity)
                    zps = ps_z.tile([cN.D, cN.DCH], f32, tag="z")
                    nc.tensor.matmul(out=zps[:], lhsT=wr_t[l][:],
                                     rhs=aggb[:], start=True, stop=False)
                    nc.tensor.matmul(out=zps[:], lhsT=wo_t[l][:],
                                     rhs=hTb[:], start=False, stop=True)
                    ep = ep_pool.tile([cN.D, cN.DCH], f32, tag="ep")
                    nc.scalar.activation(
                        out=ep[:], in_=zps[:],
                        func=mybir.ActivationFunctionType.Relu,
                        bias=br_t[l][:], scale=1.0)
                    nc.vector.tensor_add(out=hT[:, cl : cl + cN.DCH],
                                         in0=ep[:],
                                         in1=h0sl[:, j * cN.DCH : (j + 1) * cN.DCH])
                if l < cN.L - 1 and STAGE >= 5:
                    row_sb = st_pool.tile([128, cN.GB * 2 * cN.D], f16,
                                          tag="rows")
                    for b in range(cN.GB):
                        tps = ps_t.tile([128, cN.D], f32, tag="tps")
                        nc.tensor.transpose(
                            out=tps[:],
                            in_=hT[:, gb0 + b * 128 : gb0 + (b + 1) * 128],
                            identity=ident_t[: cN.D, : cN.D])
                        # write the pad half too (keeps every gathered byte
                        # initialized for the race/uninit checker)
                        nc.vector.tensor_copy(
                            out=row_sb[:, b * 2 * cN.D : (b + 1) * 2 * cN.D]
                                .rearrange("p (o d) -> p o d", d=cN.D),
                            in_=tps[:].rearrange("p (o d) -> p o d", o=1)
                                .to_broadcast([128, 2, cN.D]))
                    nc.sync.dma_start(
                        out=t_inb[l][gb0 : gb0 + GCOLS, :]
                            .rearrange("(k p) d -> p k d", p=128),
                        in_=row_sb[:].rearrange("p (k d) -> p k d",
                                                d=2 * cN.D))
                if l == cN.L - 1:
                    # fc head folded into the last layer's epilogue
                    for j in range(GCOLS // cN.DCH):
                        cl = gb0 + j * cN.DCH
                        fps = ps_fc.tile([1, cN.DCH], f32, tag="fc")
                        nc.tensor.matmul(out=fps[:], lhsT=fcw_t[:],
                                         rhs=hT[:, cl : cl + cN.DCH],
                                         start=True, stop=True)
                        osb = ep_pool.tile([1, cN.DCH], f32, tag="osb")
                        nc.scalar.activation(
                            out=osb[:], in_=fps[:],
                            func=mybir.ActivationFunctionType.Identity,
                            bias=fcb_t[:], scale=1.0)
                        nc.sync.dma_start(
                            out=t_out[:, cl : cl + cN.DCH], in_=osb[:])
                elif STAGE >= 5:
                    # one AllGather per group-set, queued when the set's
                    # last-processed group is done (emission deferred)
                    base = 0
                    for s in sets:
                        smin, smax = s[0], s[-1]
                        size = len(s) * GCOLS
                        fire = g == (smax if l % 2 == 0 else smin)
                        if fire:
                            agq.append(dict(
                                kind="AllGather",
                                op=mybir.AluOpType.bypass,
                                ins=[t_inb[l][smin * GCOLS
                                              : (smax + 1) * GCOLS, :]],
                                outs=[t_outb[l][base : base + cN.NC * size, :]],
                                replica_groups=[list(range(cN.NC))],
                            ))
                        base += cN.NC * size

    nc.compile()
    return nc


def make_in_maps(inputs, cfg, per_core):
    x = np.asarray(inputs["x"], np.float32)
    xp = np.zeros((cfg.NP, cfg.D), np.float32)
    xp[: cfg.N] = x
    iota = np.tile(
        np.concatenate([np.arange(128, dtype=np.float16) + np.float16(128 * (p % 16))
                        for p in range(31)])[None, :], (128, 1))
    ident = np.eye(128, dtype=np.float32)
    in_maps = []
    rid = rowid_map(cfg)
    xp_r = np.empty_like(xp)
    xp_r[rid] = xp                                           # table-row order
    for c in range(cfg.NC):
        xT = np.ascontiguousarray(xp[c * cfg.NSH : (c + 1) * cfg.NSH].T)
        slots = per_core[c]["slots_src"]                     # rowids
        msgs = xp_r[slots].astype(np.float16)                # [S, D]
        pre = np.ascontiguousarray(
            msgs.reshape(-1, 128, cfg.D).transpose(1, 0, 2)
                .reshape(128, -1))
        in_maps.append(dict(
            pre=pre, xT=xT,
            idx=per_core[c]["idx16"],
            dstloc=per_core[c]["dstloc"].astype(np.float16),
            iota=iota, ident=ident,
            Wr=np.ascontiguousarray(np.asarray(inputs["W_rel"], np.float32)).astype(np.float16),
            Wo=np.ascontiguousarray(np.asarray(inputs["W_root"], np.float32)).astype(np.float16),
            br=np.ascontiguousarray(np.asarray(inputs["b_rel"], np.float32)),
            fcw=np.ascontiguousarray(np.asarray(inputs["fc_w"], np.float32)),
            fcb=np.asarray(inputs["fc_b"], np.float32).reshape(1, 1),
        ))
    return in_maps


def run(inputs, cfg, trace=True):
    from concourse.bass_utils import run_bass_kernel_spmd

    src_g = np.asarray(inputs["edge_index"][0]).astype(np.int64)
    dst_g = np.asarray(inputs["edge_index"][1]).astype(np.int64)
    skeleton, per_core = make_plan(src_g, dst_g, cfg)
    fc_b_val = float(np.asarray(inputs["fc_b"]).ravel()[0])
    nc = build_nc(cfg, skeleton, fc_b_val)
    in_maps = make_in_maps(inputs, cfg, per_core)
    res = run_bass_kernel_spmd(nc, in_maps, list(range(cfg.NC)), trace=trace)
    outs = [np.asarray(res.results[c]["out"]).ravel() for c in range(cfg.NC)]
    full = np.concatenate(outs)[: cfg.N]
    return full, res


def kernel(**inputs):
    """Full inputs -> full output [N] float32."""
    cfg = FULL
    src_g = np.asarray(inputs["edge_index"][0]).astype(np.int64)
    dst_g = np.asarray(inputs["edge_index"][1]).astype(np.int64)
    skeleton, per_core = make_plan(src_g, dst_g, cfg)
    fc_b_val = float(np.asarray(inputs["fc_b"]).ravel()[0])
    nc = build_nc(cfg, skeleton, fc_b_val)
    in_maps = make_in_maps(inputs, cfg, per_core)
    from concourse.bass_utils import run_bass_kernel_spmd
    res = run_bass_kernel_spmd(nc, in_maps, list(range(cfg.NC)), trace=False)
    outs = [np.asarray(res.results[c]["out"]).ravel() for c in range(cfg.NC)]
    return np.concatenate(outs)[: cfg.N].astype(np.float32)

